# revision 16
# baseline (speedup 1.0000x reference)
"""Trainium2 Bass kernel for nn_EntanglementPropagator (gnn_message_passing).

Math (reference): for edges e=(s,d):
    out[b,d,f] = (1/norm[d]) * sum_{e->d} W[s,d,f]*cos(phase[s,d])*x[b,s,f]
               = sum_s W'[s,d,f] * x[b,s,f],
    W' = W * C,  C[s,d] = cos(phase[s,d]) * M[s,d]/norm[d]  (M = multiplicity).

i.e. per feature f an independent [B,N] x [N,N] matmul (contraction over s).

Sharding: feature dim F=256 split across 8 cores (32 f each); every core
handles all dst nodes and the full batch; no collectives.  Per core:
W f-slice 8MB + x 1MB + phase/ms 0.5MB in, 1MB out  (vs 17 MB/core for the
dst-sharded baseline).

Device pipeline per core (all heavy FP math on device):
  * C = cos(phase)*ms dense [s, d] via Sin half-angle (ACT) + DVE ops.
  * W streamed in (kb, f-chunk) pieces, f-major [s, f, d]; DVE fuses the
    C-scale (broadcast over f) with the fp32->bf16 cast.
  * PE: W' stationary - each (f, d-block) is a contiguous 128-column bf16
    weight load (FWL-eligible), x[s, b] moving (32 cols).  All 64
    (f, d-block) PSUM accumulators stay open across the two kb passes
    (8 KB of the 16 KB PSUM per buffer), so each weight tile is loaded
    exactly once.
  * PSUM drains [128, 32] alternate ACT/DVE; output [d-part, f, b] goes out
    contiguously, host restores [b, d, f] (pure layout).

The kernel is HBM-stream-bound on W (~23.6us of the ~30us total at the
measured 347 GB/s single-queue rate); DVE (~21us) hides under it.
"""

import numpy as np

import concourse.mybir as mybir
import concourse.tile as tile
from concourse import bacc
from concourse.bass_utils import run_bass_kernel_spmd

N = 256          # nodes
F = 256          # feature dim
B = 32           # batch
N_CORES = 8
FS = F // N_CORES        # f-slice per core = 32
KB = 2                   # source-node partition blocks (s: 2 x 128)
FC = 8                   # f-planes per W piece (1MB fp32 pieces)
F32 = mybir.dt.float32
BF16 = mybir.dt.bfloat16

HALF_PI = float(np.pi / 2.0)


def build_body(tc, wt, phm, xs, out, opts=()):
    opts = set(opts)
    """wt  [KB, 128, FS, N] DRAM - W'[s, f, d] layout (host-transposed)
    phm [2, N, N]         DRAM - phase and M/norm scale (dense (s,d))
    xs  [N, FS, B]        DRAM - node features, f-major
    out [KB, 128, FS, B]  DRAM - this core's output, d on partitions
    """
    nc = tc.nc

    with (
        tc.tile_pool(name="prep", bufs=1) as prep,
        tc.tile_pool(name="wpool",
                     bufs=3 if "buf3" in opts else 2) as wpool,
        tc.tile_pool(name="opool", bufs=1) as opool,
        tc.tile_pool(name="ppool", bufs=8, space="PSUM") as ppool,
    ):
        # phase/ms [2, (kb p), d] -> [128, 2, kb, d]
        phm_t = prep.tile([128, 2, KB, N], F32, tag="phm")
        phm_ring = nc.scalar if "phmact" in opts else nc.sync
        phm_ring.dma_start(out=phm_t,
                           in_=phm.rearrange("t (k p) d -> p t k d", k=KB))
        # C = cos(phase)*ms via cos(x) = 2*sin^2(x/2 - pi/2) - 1
        bias_t = prep.tile([128, 1], F32, tag="bias")
        nc.vector.memset(bias_t, -HALF_PI)
        c_t = prep.tile([128, KB, N], F32, tag="c")
        nc.scalar.activation(out=c_t, in_=phm_t[:, 0],
                             func=mybir.ActivationFunctionType.Sin,
                             bias=bias_t, scale=0.5)
        nc.vector.tensor_mul(out=c_t, in0=c_t, in1=c_t)
        nc.vector.tensor_scalar(out=c_t, in0=c_t, scalar1=2.0, scalar2=-1.0,
                                op0=mybir.AluOpType.mult,
                                op1=mybir.AluOpType.add)
        nc.vector.tensor_mul(out=c_t, in0=c_t, in1=phm_t[:, 1])

        # x [N, FS, B] -> [128, kb, f, b], cast bf16
        xs_f = prep.tile([128, KB, FS, B], F32, tag="xsf")
        xs_ring = nc.scalar if "tailfix" in opts else nc.sync
        xs_ring.dma_start(out=xs_f,
                          in_=xs.rearrange("(k p) f b -> p k f b", k=KB))
        xs_t = prep.tile([128, KB, FS, B], BF16, tag="xsb")
        if "actxcast" in opts:
            nc.scalar.copy(out=xs_t, in_=xs_f)
        else:
            nc.vector.tensor_copy(out=xs_t, in_=xs_f)

        out_sb = opool.tile([128, KB, FS, B], F32)
        if "dmaonly" in opts:
            nc.vector.memset(out_sb, 0.0)

        fc = 16 if "fc16" in opts else (4 if "fc4" in opts else FC)
        if "wdec" in opts:
            sizes = [16, 8, 4, 4]
        else:
            sizes = [fc] * (FS // fc)
        f0s = [sum(sizes[:i]) for i in range(len(sizes))]
        fcmax = max(sizes)
        for ci, (f0, fc) in enumerate(zip(f0s, sizes)):
            wb = []
            for kb in range(KB):
                wp = wpool.tile([128, fcmax, N], F32, tag=f"wp{kb}")
                wp = wp[:, :fc, :]
                ring = nc.scalar if ("w2r" in opts and kb == 1) else nc.sync
                ring.dma_start(out=wp, in_=wt[kb, :, f0:f0 + fc, :])
                if "dmaonly" not in opts:
                    wbk = wpool.tile([128, fcmax, N], BF16, tag=f"wb{kb}")
                    wbk = wbk[:, :fc, :]
                    if "noscale" in opts:
                        # timing probe: cast without the C multiply
                        nc.vector.tensor_copy(out=wbk, in_=wp)
                    elif "gpsplit" in opts:
                        h = fc // 2
                        nc.vector.tensor_mul(
                            out=wbk[:, :h, :], in0=wp[:, :h, :],
                            in1=c_t[:, kb, None, :].broadcast_to([128, h, N]))
                        nc.gpsimd.tensor_mul(
                            out=wbk[:, h:, :], in0=wp[:, h:, :],
                            in1=c_t[:, kb, None, :].broadcast_to([128, h, N]))
                    else:
                        nc.vector.tensor_mul(
                            out=wbk, in0=wp,
                            in1=c_t[:, kb, None, :].broadcast_to([128, fc, N]))
                    wb.append(wbk)
            if "dmaonly" in opts:
                continue
            for fi in range(fc):
                f = f0 + fi
                for c in range(KB):
                    ps = ppool.tile([128, B], F32)
                    for kb in range(KB):
                        nc.tensor.matmul(
                            ps,
                            lhsT=wb[kb][:, fi, c * 128:(c + 1) * 128],
                            rhs=xs_t[:, kb, f, :],
                            start=(kb == 0), stop=(kb == KB - 1))
                    if f % 2 == 0:
                        nc.scalar.copy(out=out_sb[:, c, f, :], in_=ps)
                    else:
                        nc.vector.tensor_copy(out=out_sb[:, c, f, :], in_=ps)
            if "tailfix" in opts:
                for c in range(KB):
                    nc.scalar.dma_start(
                        out=out[c, :, f0:f0 + fc, :].rearrange(
                            "p f b -> p (f b)"),
                        in_=out_sb[:, c, f0:f0 + fc, :])
        if "tailfix" not in opts:
            for c in range(KB):
                nc.scalar.dma_start(out=out[c].rearrange("p f b -> p (f b)"),
                                    in_=out_sb[:, c])


def build_program(n_repeat=1, loop_k=None, opts=(), ncores=N_CORES):
    nc = bacc.Bacc("TRN2", target_bir_lowering=False, debug=False,
                   num_devices=ncores)
    wt = nc.dram_tensor("wt", [KB, 128, FS, N], F32,
                        kind="ExternalInput").ap()
    phm = nc.dram_tensor("phm", [2, N, N], F32, kind="ExternalInput").ap()
    xs = nc.dram_tensor("xs", [N, FS, B], F32, kind="ExternalInput").ap()
    out = nc.dram_tensor("out", [KB, 128, FS, B], F32,
                         kind="ExternalOutput").ap()

    with tile.TileContext(nc) as tc:
        if loop_k is not None:
            with tc.For_i(0, loop_k, 1):
                for _ in range(n_repeat):
                    build_body(tc, wt, phm, xs, out, opts)
        else:
            for _ in range(n_repeat):
                build_body(tc, wt, phm, xs, out, opts)
    nc.compile()
    return nc


def host_prep(src, dst):
    """ms[s,d] = multiplicity/out-degree-norm from the integer edge tensors."""
    src = np.asarray(src).astype(np.int64)
    dst = np.asarray(dst).astype(np.int64)
    counts = np.bincount(src, minlength=N).astype(np.float64)
    norm = np.maximum(counts, 1.0)
    mult = np.bincount(src * N + dst, minlength=N * N).astype(np.float64)
    ms = (mult.reshape(N, N) / norm[None, :]).astype(np.float32)
    return ms


_PROGRAM_CACHE = {}


def get_program(n_repeat=1, loop_k=None, opts=(), ncores=N_CORES):
    key = (n_repeat, loop_k, tuple(opts), ncores)
    if key not in _PROGRAM_CACHE:
        _PROGRAM_CACHE[key] = build_program(n_repeat, loop_k, opts, ncores)
    return _PROGRAM_CACHE[key]


def make_in_maps(node_features, W, phase, src, dst):
    node_features = np.asarray(node_features, dtype=np.float32)
    W = np.asarray(W, dtype=np.float32)
    phase = np.asarray(phase, dtype=np.float32)
    ms = host_prep(src, dst)
    phm = np.ascontiguousarray(np.stack([phase, ms], axis=0))  # [2, N, N]
    xt = np.ascontiguousarray(node_features.transpose(1, 2, 0))  # [N, F, B]
    in_maps = []
    for c in range(N_CORES):
        fsl = slice(c * FS, (c + 1) * FS)
        # W[s, d, f-slice] -> [kb, p, f, d]
        wc = np.ascontiguousarray(
            W[:, :, fsl].transpose(0, 2, 1).reshape(KB, 128, FS, N))
        in_maps.append({
            "wt": wc,
            "phm": phm,
            "xs": np.ascontiguousarray(xt[:, fsl, :]),
        })
    return in_maps


DEFAULT_OPTS = ("tailfix", "wdec")


def kernel(node_features, W, phase, src, dst):
    nc = get_program(1, None, DEFAULT_OPTS)
    in_maps = make_in_maps(node_features, W, phase, src, dst)
    res = run_bass_kernel_spmd(nc, in_maps, list(range(N_CORES)))
    out = np.empty((B, N, F), np.float32)
    for c in range(N_CORES):
        oc = res.results[c]["out"]                 # [KB(c), 128(p), FS, B]
        # d = kb*128 + p ;  [c, p, f, b] -> [b, (c p), f]
        out[:, :, c * FS:(c + 1) * FS] = (
            oc.transpose(3, 0, 1, 2).reshape(B, N, FS))
    return out
